# revision 1
# baseline (speedup 1.0000x reference)
"""Trainium2 Bass kernel for the prompted-GCN pipeline (gnn_message_passing).

Data-parallel over the graph batch: 8 NeuronCores x 8 graphs each.

Sharding/layout choice (host side, per the free-choice sharding contract):
edges are stored destination-sorted per graph (CSR-style layout, standard for
GNN inputs). The host marshals index tensors (int16 casts, wrapped layouts,
run-boundary ranks derived from the sorted layout) and folds the
graph-independent prompt-token stream into constants. All x/edge VALUE
computation (matmuls, masks, degrees, normalization, message gather,
prefix-scan segment reduction, pooling, softmax) runs on device.

Device algorithm per graph:
  xT = transpose(x); Z = tokens @ xT; M_cr = (Z >= logit(0.1))
  deg = boundary-rank difference (from dst-sorted run boundaries)
  deg_node = 1 + deg + colsum(M_cr); inv = rsqrt(deg_node)
  h' = inv * (x @ W1)  -> DRAM table
  G = h'[src_sorted]   (dma_gather, rank-major-per-partition layout)
  P = prefix-scan(G)   -> DRAM table (rows 1..E; row 0 = 0)
  y[n] = P[end_n] - P[start_n]   (dma_gather at run boundaries)
  hn1a = lrelu(inv*(y + cross1raw + h') + b1)
  layer-2 collapses to reductions:
    dvec = sum_n inv2[n]*hn1a[n];  g = inv*hn1a -> DRAM
    cvec = sum_R inv[dst_R]*g[src_R]  (two gathers + fused mul-reduce)
    zvec = sum_n inv[n]*cross2raw[n]
    out = softmax(((cvec+dvec)@W2 + zvec + N*b2 + tok_sum2) @ Wa/(T+N) + ba)
"""

import sys

sys.path.insert(0, '/opt/trn_rl_repo')
import antenv  # noqa: E402

if '/opt/trn_rl_repo/antenv' not in antenv.__path__:
    antenv.__path__.append('/opt/trn_rl_repo/antenv')

import numpy as np  # noqa: E402

B, N, E, F, H, T, C = 64, 1024, 16384, 128, 64, 10, 2
NCORES = 8
BLOC = B // NCORES
NEG_SLOPE = 0.01
INNER_PRUNE, CROSS_PRUNE = 0.3, 0.1
THR_CROSS = float(np.log(CROSS_PRUNE / (1.0 - CROSS_PRUNE)))  # sigmoid(z)>=p  <=>  z>=logit(p)

_CACHE = {}


def _token_constants(tokens, W1, b1, W2, b2, Wa, ba):
    """Fold the graph-independent prompt-token stream (all f32 numpy)."""
    t = tokens.astype(np.float32)

    def sigmoid(v):
        return (1.0 / (1.0 + np.exp(-v.astype(np.float32)))).astype(np.float32)

    M_in = (sigmoid(t @ t.T) >= INNER_PRUNE).astype(np.float32)
    deg_tok = 1.0 + M_in.sum(0)
    inv_tok = (1.0 / np.sqrt(deg_tok)).astype(np.float32)
    norm_in = M_in * inv_tok[:, None] * inv_tok[None, :]
    ht1lin = t @ W1
    out_tok1 = norm_in @ ht1lin + ht1lin * (1.0 / deg_tok)[:, None] + b1
    ht1a = np.where(out_tok1 >= 0, out_tok1, NEG_SLOPE * out_tok1).astype(np.float32)
    ht2lin = ht1a @ W2
    out_tok2 = norm_in @ ht2lin + ht2lin * (1.0 / deg_tok)[:, None] + b2
    tok_sum2 = out_tok2.sum(0).astype(np.float32)
    cT1 = inv_tok[:, None] * ht1lin
    cT2 = inv_tok[:, None] * ht2lin
    cT12 = np.concatenate([cT1, cT2], axis=1).astype(np.float32)  # [10, 128]
    return cT12, tok_sum2


def _wrap16(vals):
    """Edge-position layout for SWDGE idx tensors: position q -> [q%16, q//16]."""
    n = vals.shape[0]
    w = np.zeros((16, n // 16), np.int16)
    q = np.arange(n)
    w[q % 16, q // 16] = vals
    return w


def _host_graph_prep(src, dst):
    """Per-graph host marshalling: dst-sort + wrapped idx layouts + boundaries."""
    order = np.argsort(dst, kind='stable')
    src_s = src[order].astype(np.int64)
    dst_s = dst[order].astype(np.int64)
    # position q holds sorted-rank R(q) = (q%128)*128 + q//128 so that the
    # gather slot (q%128, q//128) = (R//128, R%128): rank-major per partition.
    q = np.arange(E)
    Rq = (q % 128) * 128 + q // 128
    srcw = np.tile(_wrap16(src_s[Rq].astype(np.int16)), (8, 1))  # [128, 1024]
    dstw = np.tile(_wrap16(dst_s[Rq].astype(np.int16)), (8, 1))
    # run boundaries in rank space (P table is offset by one: P[r]=sum of first r)
    idx_end = np.searchsorted(dst_s, np.arange(N), side='right').astype(np.int16)
    idx_start = np.searchsorted(dst_s, np.arange(N), side='left').astype(np.int16)
    nn = np.arange(N)
    w_end = np.zeros((16, 64), np.int16)
    w_start = np.zeros((16, 64), np.int16)
    w_end[nn % 16, nn // 16] = idx_end
    w_start[nn % 16, nn // 16] = idx_start
    bidx1 = np.tile(w_end, (8, 1))   # [128, 64]
    bidx2 = np.tile(w_start, (8, 1))
    # col layout: [p, t] = node t*128+p
    bc1 = np.ascontiguousarray(idx_end.reshape(8, 128).T)
    bc2 = np.ascontiguousarray(idx_start.reshape(8, 128).T)
    return srcw, dstw, bidx1, bidx2, bc1, bc2


def _build_program(bloc):
    from concourse import bacc, tile, mybir

    fp32 = mybir.dt.float32
    bf16 = mybir.dt.bfloat16
    i16 = mybir.dt.int16
    AF = mybir.ActivationFunctionType
    ALU = mybir.AluOpType

    nc = bacc.Bacc("TRN2", target_bir_lowering=False, debug=True,
                   dynamic_dma_scratch_size=16384, num_swdge_queues=4)

    x_p = nc.declare_dram_parameter("x", [bloc, N, F], fp32, isOutput=False)
    srcw_p = nc.declare_dram_parameter("srcw", [bloc, 128, E // 16], i16, isOutput=False)
    dstw_p = nc.declare_dram_parameter("dstw", [bloc, 128, E // 16], i16, isOutput=False)
    b1x_p = nc.declare_dram_parameter("bidx1", [bloc, 128, 64], i16, isOutput=False)
    b2x_p = nc.declare_dram_parameter("bidx2", [bloc, 128, 64], i16, isOutput=False)
    bc1_p = nc.declare_dram_parameter("bcol1", [bloc, 128, 8], i16, isOutput=False)
    bc2_p = nc.declare_dram_parameter("bcol2", [bloc, 128, 8], i16, isOutput=False)
    W1_p = nc.declare_dram_parameter("W1", [F, H], fp32, isOutput=False)
    tokT_p = nc.declare_dram_parameter("tokT", [F, T], fp32, isOutput=False)
    cT12_p = nc.declare_dram_parameter("cT12", [T, 2 * H], fp32, isOutput=False)
    b1t_p = nc.declare_dram_parameter("b1t", [128, H], fp32, isOutput=False)
    W2_p = nc.declare_dram_parameter("W2", [H, H], fp32, isOutput=False)
    const64_p = nc.declare_dram_parameter("c64", [H, 1], fp32, isOutput=False)
    Wa_p = nc.declare_dram_parameter("Wa", [H, C], fp32, isOutput=False)
    bat_p = nc.declare_dram_parameter("bat", [bloc, C], fp32, isOutput=False)
    ident_p = nc.declare_dram_parameter("ident", [128, 128], fp32, isOutput=False)
    out_p = nc.declare_dram_parameter("out", [bloc, C], fp32, isOutput=True)

    htab = [nc.dram_tensor(f"htab{g}", [N, H], fp32) for g in range(bloc)]
    gtab = [nc.dram_tensor(f"gtab{g}", [N, H], fp32) for g in range(bloc)]
    itab = [nc.dram_tensor(f"itab{g}", [N, H], fp32) for g in range(bloc)]
    ptab = [nc.dram_tensor(f"ptab{g}", [E + 1, H], fp32) for g in range(bloc)]

    with tile.TileContext(nc) as tc:
        with (
            tc.tile_pool(name="const", bufs=1) as cpool,
            tc.tile_pool(name="idx", bufs=2) as ipool,
            tc.tile_pool(name="xp", bufs=2) as xpool,
            tc.tile_pool(name="big", bufs=2) as bigpool,
            tc.tile_pool(name="scan", bufs=1) as scanpool,
            tc.tile_pool(name="work", bufs=2) as wpool,
            tc.tile_pool(name="ps", bufs=2, space="PSUM") as pspool,
            tc.tile_pool(name="psm", bufs=1, space="PSUM") as psmpool,
            tc.tile_pool(name="psb", bufs=1, space="PSUM") as psbpool,
        ):
            # ---- constants ----
            W1_t = cpool.tile([F, H], fp32)
            nc.sync.dma_start(out=W1_t[:], in_=W1_p[:])
            tokT_t = cpool.tile([F, T], fp32)
            nc.sync.dma_start(out=tokT_t[:], in_=tokT_p[:])
            cT12_t = cpool.tile([T, 2 * H], fp32)
            nc.sync.dma_start(out=cT12_t[:], in_=cT12_p[:])
            cT12_b = cpool.tile([T, 2 * H], bf16)
            nc.vector.tensor_copy(cT12_b[:], cT12_t[:])
            b1t_t = cpool.tile([128, H], fp32)
            nc.sync.dma_start(out=b1t_t[:], in_=b1t_p[:])
            W2_t = cpool.tile([H, H], fp32)
            nc.sync.dma_start(out=W2_t[:], in_=W2_p[:])
            c64_t = cpool.tile([H, 1], fp32)
            nc.sync.dma_start(out=c64_t[:], in_=const64_p[:])
            Wa_t = cpool.tile([H, C], fp32)
            nc.sync.dma_start(out=Wa_t[:], in_=Wa_p[:])
            bat_t = cpool.tile([bloc, C], fp32)
            nc.sync.dma_start(out=bat_t[:], in_=bat_p[:])
            ident_t = cpool.tile([128, 128], fp32)
            nc.sync.dma_start(out=ident_t[:], in_=ident_p[:])
            ones10 = cpool.tile([T, 1], bf16)
            nc.vector.memset(ones10[:], 1.0)
            ones128 = cpool.tile([128, 1], fp32)
            nc.vector.memset(ones128[:], 1.0)
            onesH = cpool.tile([128, H], fp32)
            nc.vector.memset(onesH[:], 1.0)
            zrow = cpool.tile([1, H], fp32)
            nc.vector.memset(zrow[:], 0.0)
            for g in range(bloc):
                nc.sync.dma_start(out=ptab[g][0:1, :], in_=zrow[:])

            # head accumulators
            SD_T = cpool.tile([H, bloc], fp32)
            Z_T = cpool.tile([H, bloc], fp32)

            for g in range(bloc):
                # ---- loads ----
                srcw_t = ipool.tile([128, E // 16], i16, tag="srcw")
                nc.sync.dma_start(out=srcw_t[:], in_=srcw_p[g])
                dstw_t = ipool.tile([128, E // 16], i16, tag="dstw")
                nc.sync.dma_start(out=dstw_t[:], in_=dstw_p[g])
                b1x_t = ipool.tile([128, 64], i16, tag="b1x")
                nc.sync.dma_start(out=b1x_t[:], in_=b1x_p[g])
                b2x_t = ipool.tile([128, 64], i16, tag="b2x")
                nc.sync.dma_start(out=b2x_t[:], in_=b2x_p[g])

                x_t = xpool.tile([128, 8, F], fp32, tag="x")
                nc.sync.dma_start(out=x_t[:], in_=x_p[g].rearrange("(t p) f -> p t f", p=128))

                # ---- transpose x -> xT [128f, 8*128n] ----
                xT = xpool.tile([F, 8, 128], fp32, tag="xT")
                for t in range(8):
                    ps = pspool.tile([128, 128], fp32, tag="work")
                    nc.tensor.transpose(ps[:], x_t[:, t, :], ident_t[:])
                    nc.scalar.copy(xT[:, t, :], ps[:])

                # ---- M_cr mask [10, 1024] bf16 ----
                mcr_ps = psmpool.tile([T, N], fp32, tag="mcr")
                for hblk in range(2):
                    nc.tensor.matmul(
                        mcr_ps[:, hblk * 512:(hblk + 1) * 512],
                        tokT_t[:],
                        xT[:].rearrange("p t n -> p (t n)")[:, hblk * 512:(hblk + 1) * 512],
                        start=True, stop=True)
                mask_b = wpool.tile([T, N], bf16, tag="mask")
                nc.vector.tensor_scalar(mask_b[:], mcr_ps[:], THR_CROSS, None, ALU.is_ge)

                # ---- M_cr column sums -> [128, 8] (per node) ----
                mcrcol_ps = psbpool.tile([128, 8], fp32, tag="misc")
                for t in range(8):
                    nc.tensor.matmul(mcrcol_ps[:, t:t + 1],
                                     mask_b[:, t * 128:(t + 1) * 128],
                                     ones10[:], start=True, stop=True)

                # ---- degrees from boundary ranks ----
                b1c = wpool.tile([128, 8], i16, tag="b1c")
                nc.sync.dma_start(out=b1c[:], in_=bc1_p[g])
                b2c = wpool.tile([128, 8], i16, tag="b2c")
                nc.sync.dma_start(out=b2c[:], in_=bc2_p[g])
                dcol_i = wpool.tile([128, 8], i16, tag="dcoli")
                nc.vector.tensor_tensor(dcol_i[:], b1c[:], b2c[:], ALU.subtract)
                dcol = wpool.tile([128, 8], fp32, tag="dcol")
                nc.vector.tensor_copy(dcol[:], dcol_i[:])

                dn = wpool.tile([128, 8], fp32, tag="dn")
                nc.vector.tensor_tensor(dn[:], dcol[:], mcrcol_ps[:], ALU.add)
                nc.vector.tensor_scalar_add(dn[:], dn[:], 1.0)
                inv2c = wpool.tile([128, 8], fp32, tag="inv2c")
                nc.vector.reciprocal(inv2c[:], dn[:])
                invc = wpool.tile([128, 8], fp32, tag="invc")
                nc.scalar.sqrt(invc[:], inv2c[:])

                # ---- h' = inv * (x @ W1) -> htab ----
                h1 = xpool.tile([128, 8, H], fp32, tag="h1")
                for t in range(8):
                    hps = pspool.tile([128, H], fp32, tag="work")
                    nc.tensor.matmul(hps[:], xT[:, t, :], W1_t[:], start=True, stop=True)
                    nc.scalar.activation(h1[:, t, :], hps[:], AF.Copy,
                                         scale=invc[:, t:t + 1])
                nc.sync.dma_start(out=htab[g][:].rearrange("(t p) f -> p t f", p=128),
                                  in_=h1[:])

                # ---- cross terms [128n, 128] per tile ----
                crs = xpool.tile([128, 8, 2 * H], fp32, tag="crs")
                for t in range(8):
                    cps = pspool.tile([128, 2 * H], fp32, tag="work")
                    nc.tensor.matmul(cps[:], mask_b[:, t * 128:(t + 1) * 128],
                                     cT12_b[:], start=True, stop=True)
                    nc.scalar.copy(crs[:, t, :], cps[:])

                # ---- gather messages G = h'[src_sorted] ----
                G = bigpool.tile([128, 128, H], fp32, tag="GA")
                nc.gpsimd.dma_gather(G[:, 0:64, :], htab[g][:], srcw_t[:, 0:512],
                                     E // 2, E // 2, H, queue_num=0, single_packet=False)
                nc.gpsimd.dma_gather(G[:, 64:128, :], htab[g][:], srcw_t[:, 512:1024],
                                     E // 2, E // 2, H, queue_num=1, single_packet=False)

                # ---- prefix scan along ranks (per partition, per feature) ----
                P = scanpool.tile([128, 128, H], fp32, tag="GB")
                for f in range(H):
                    nc.vector.tensor_tensor_scan(
                        P[:, :, f:f + 1].squeeze(), G[:, :, f:f + 1].squeeze(),
                        G[:, :, f:f + 1].squeeze(), 0.0,
                        op0=ALU.add, op1=ALU.bypass)
                nc.sync.dma_start(
                    out=ptab[g][1:E + 1, :].rearrange("(p j) f -> p j f", p=128),
                    in_=P[:])

                # ---- segment sums via boundary gathers ----
                yb1 = wpool.tile([128, 8, H], fp32, tag="yb1")
                nc.gpsimd.dma_gather(yb1[:], ptab[g][:], b1x_t[:], N, N, H,
                                     queue_num=2, single_packet=False)
                yb2 = wpool.tile([128, 8, H], fp32, tag="yb2")
                nc.gpsimd.dma_gather(yb2[:], ptab[g][:], b2x_t[:], N, N, H,
                                     queue_num=3, single_packet=False)

                # ---- hn1a = lrelu(inv*(y + cross1 + h') + b1) ----
                hn = xpool.tile([128, 8, H], fp32, tag="hn")
                nc.vector.tensor_tensor(hn[:], yb1[:], yb2[:], ALU.subtract)
                nc.vector.tensor_tensor(hn[:], hn[:], crs[:, :, 0:H], ALU.add)
                nc.vector.tensor_tensor(hn[:], hn[:], h1[:], ALU.add)
                for t in range(8):
                    nc.scalar.activation(hn[:, t, :], hn[:, t, :], AF.Copy,
                                         scale=invc[:, t:t + 1])
                    nc.vector.tensor_tensor(hn[:, t, :], hn[:, t, :], b1t_t[:], ALU.add)
                hnm = xpool.tile([128, 8, H], fp32, tag="hnm")
                nc.vector.tensor_scalar_mul(hnm[:], hn[:], NEG_SLOPE)
                nc.vector.tensor_tensor(hn[:], hn[:], hnm[:], ALU.max)

                # ---- dvec += sum_n inv2[n] hn1a[n]; g-table; inv-table ----
                dv_ps = psbpool.tile([H, 1], fp32, tag="dv")
                zv_ps = psbpool.tile([H, 1], fp32, tag="zv")
                gsb = xpool.tile([128, 8, H], fp32, tag="gsb")
                irep = xpool.tile([128, 8, H], fp32, tag="irep")
                for t in range(8):
                    nc.tensor.matmul(dv_ps[:], hn[:, t, :], inv2c[:, t:t + 1],
                                     start=(t == 0), stop=(t == 7))
                    nc.tensor.matmul(zv_ps[:], crs[:, t, H:2 * H], invc[:, t:t + 1],
                                     start=(t == 0), stop=(t == 7))
                    nc.scalar.activation(gsb[:, t, :], hn[:, t, :], AF.Copy,
                                         scale=invc[:, t:t + 1])
                    nc.scalar.activation(irep[:, t, :], onesH[:], AF.Copy,
                                         scale=invc[:, t:t + 1])
                nc.sync.dma_start(out=gtab[g][:].rearrange("(t p) f -> p t f", p=128),
                                  in_=gsb[:])
                nc.sync.dma_start(out=itab[g][:].rearrange("(t p) f -> p t f", p=128),
                                  in_=irep[:])

                # ---- c-pass: cvec = sum_R inv[dst_R] * g[src_R] ----
                GG = bigpool.tile([128, 128, H], fp32, tag="GA")
                nc.gpsimd.dma_gather(GG[:, 0:64, :], gtab[g][:], srcw_t[:, 0:512],
                                     E // 2, E // 2, H, queue_num=0, single_packet=False)
                nc.gpsimd.dma_gather(GG[:, 64:128, :], gtab[g][:], srcw_t[:, 512:1024],
                                     E // 2, E // 2, H, queue_num=1, single_packet=False)
                GI = scanpool.tile([128, 128, H], fp32, tag="GB")
                nc.gpsimd.dma_gather(GI[:, 0:64, :], itab[g][:], dstw_t[:, 0:512],
                                     E // 2, E // 2, H, queue_num=2, single_packet=False)
                nc.gpsimd.dma_gather(GI[:, 64:128, :], itab[g][:], dstw_t[:, 512:1024],
                                     E // 2, E // 2, H, queue_num=3, single_packet=False)
                nc.vector.tensor_tensor(GG[:], GG[:], GI[:], ALU.mult)
                csum = wpool.tile([128, H], fp32, tag="csum")
                nc.vector.tensor_reduce(
                    csum[:].rearrange("p (f o) -> p f o", o=1),
                    GG[:].rearrange("p j f -> p f j"),
                    mybir.AxisListType.X, ALU.add)
                cv_ps = psbpool.tile([H, 1], fp32, tag="misc")
                nc.tensor.matmul(cv_ps[:], csum[:], ones128[:], start=True, stop=True)

                # ---- per-graph head columns ----
                dvs = wpool.tile([H, 1], fp32, tag="dvs")
                nc.scalar.copy(dvs[:], dv_ps[:])
                nc.vector.tensor_tensor(SD_T[:, g:g + 1], cv_ps[:], dvs[:], ALU.add)
                nc.scalar.copy(Z_T[:, g:g + 1], zv_ps[:])

            # ---- batched head ----
            emb_ps = psbpool.tile([H, bloc], fp32, tag="misc")
            nc.tensor.matmul(emb_ps[:], W2_t[:], SD_T[:], start=True, stop=True)
            embT = cpool.tile([H, bloc], fp32)
            nc.vector.tensor_tensor(embT[:], emb_ps[:], Z_T[:], ALU.add)
            nc.vector.tensor_scalar(embT[:], embT[:], c64_t[:], None, ALU.add)
            lg_ps = psbpool.tile([bloc, C], fp32, tag="misc")
            nc.tensor.matmul(lg_ps[:], embT[:], Wa_t[:], start=True, stop=True)
            lg = cpool.tile([bloc, C], fp32)
            nc.vector.tensor_tensor(lg[:], lg_ps[:], bat_t[:], ALU.add)
            mx = cpool.tile([bloc, 1], fp32)
            nc.vector.tensor_reduce(mx[:], lg[:], mybir.AxisListType.X, ALU.max)
            nmx = cpool.tile([bloc, 1], fp32)
            nc.vector.tensor_scalar_mul(nmx[:], mx[:], -1.0)
            ex = cpool.tile([bloc, C], fp32)
            nc.scalar.activation(ex[:], lg[:], AF.Exp, bias=nmx[:])
            sm = cpool.tile([bloc, 1], fp32)
            nc.vector.tensor_reduce(sm[:], ex[:], mybir.AxisListType.X, ALU.add)
            rs = cpool.tile([bloc, 1], fp32)
            nc.vector.reciprocal(rs[:], sm[:])
            outt = cpool.tile([bloc, C], fp32)
            nc.vector.tensor_scalar(outt[:], ex[:], rs[:], None, ALU.mult)
            nc.sync.dma_start(out=out_p[:], in_=outt[:])

    nc.compile()
    return nc


def _get_program(bloc=BLOC):
    if bloc not in _CACHE:
        _CACHE[bloc] = _build_program(bloc)
    return _CACHE[bloc]


def build_in_maps(x, tokens, W1, b1, W2, b2, Wa, ba, edge_src, edge_dst,
                  ncores=NCORES, bloc=BLOC):
    x = np.asarray(x, np.float32)
    cT12, tok_sum2 = _token_constants(
        np.asarray(tokens, np.float32), np.asarray(W1, np.float32),
        np.asarray(b1, np.float32), np.asarray(W2, np.float32),
        np.asarray(b2, np.float32), np.asarray(Wa, np.float32),
        np.asarray(ba, np.float32))
    const64 = (N * np.asarray(b2, np.float32) + tok_sum2).reshape(H, 1)
    shared = {
        "W1": np.asarray(W1, np.float32),
        "tokT": np.ascontiguousarray(np.asarray(tokens, np.float32).T),
        "cT12": cT12,
        "b1t": np.tile(np.asarray(b1, np.float32)[None, :], (128, 1)),
        "W2": np.asarray(W2, np.float32),
        "c64": const64,
        "Wa": (np.asarray(Wa, np.float32) / float(T + N)),
        "bat": np.tile(np.asarray(ba, np.float32)[None, :], (bloc, 1)),
        "ident": np.eye(128, dtype=np.float32),
    }
    in_maps = []
    for c in range(ncores):
        srcw = np.zeros((bloc, 128, E // 16), np.int16)
        dstw = np.zeros((bloc, 128, E // 16), np.int16)
        bx1 = np.zeros((bloc, 128, 64), np.int16)
        bx2 = np.zeros((bloc, 128, 64), np.int16)
        bc1 = np.zeros((bloc, 128, 8), np.int16)
        bc2 = np.zeros((bloc, 128, 8), np.int16)
        for g in range(bloc):
            gi = c * bloc + g
            srcw[g], dstw[g], bx1[g], bx2[g], bc1[g], bc2[g] = _host_graph_prep(
                np.asarray(edge_src[gi]), np.asarray(edge_dst[gi]))
        m = dict(shared)
        m["x"] = np.ascontiguousarray(x[c * bloc:(c + 1) * bloc])
        m["srcw"] = srcw
        m["dstw"] = dstw
        m["bidx1"] = bx1
        m["bidx2"] = bx2
        m["bcol1"] = bc1
        m["bcol2"] = bc2
        in_maps.append(m)
    return in_maps


def kernel(x, tokens, W1, b1, W2, b2, Wa, ba, edge_src, edge_dst):
    from concourse.bass_utils import run_bass_kernel_spmd

    nc = _get_program()
    in_maps = build_in_maps(x, tokens, W1, b1, W2, b2, Wa, ba, edge_src, edge_dst)
    res = run_bass_kernel_spmd(nc, in_maps, list(range(NCORES)))
    out = np.concatenate([res.results[c]["out"] for c in range(NCORES)], axis=0)
    return out.astype(np.float32)



# revision 2
# speedup vs baseline: 9.3459x; 9.3459x over previous
"""Trainium2 Bass kernel for the prompted-GCN pipeline (gnn_message_passing).

Data-parallel over the graph batch: 8 NeuronCores x 8 graphs each.

Sharding/layout choice (host side, per the free-choice sharding contract):
the host re-encodes each graph's edge list as a dense count matrix
A[src, dst] (pure index marshalling, like CSR conversion) and folds the
graph-independent prompt-token stream into constants. All x/edge VALUE
computation (matmuls, masks, degrees, normalization, aggregation, pooling,
softmax) runs on device.

Device algorithm per graph (A replaces all gathers/scatter):
  xT = transpose(x); Z = tokens @ xT; M_cr = (Z >= logit(0.1))
  deg_node = 1 + indeg + colsum(M_cr); inv = rsqrt(deg_node)
  h1 = inv * (x @ W1)                     [SBUF only]
  y = A^T @ h1      (dense bf16 matmul, 8x8 tile grid, PSUM accumulate)
  hn1a = lrelu(inv*(y + cross1raw + h1) + b1)
  layer-2 collapses to reductions:
    dvec = sum_n inv2[n]*hn1a[n];  g2 = inv*hn1a
    agg2 = A^T @ g2;  cvec = sum_n inv[n]*agg2[n]
    zvec = sum_n inv[n]*cross2raw[n]
    out = softmax(((cvec+dvec)@W2 + zvec + N*b2 + tok_sum2) @ Wa/(T+N) + ba)
"""

import sys

sys.path.insert(0, '/opt/trn_rl_repo')
import antenv  # noqa: E402

if '/opt/trn_rl_repo/antenv' not in antenv.__path__:
    antenv.__path__.append('/opt/trn_rl_repo/antenv')

import numpy as np  # noqa: E402
import ml_dtypes  # noqa: E402

B, N, E, F, H, T, C = 64, 1024, 16384, 128, 64, 10, 2
NCORES = 8
BLOC = B // NCORES
NEG_SLOPE = 0.01
INNER_PRUNE, CROSS_PRUNE = 0.3, 0.1
THR_CROSS = float(np.log(CROSS_PRUNE / (1.0 - CROSS_PRUNE)))  # sigmoid(z)>=p  <=>  z>=logit(p)
BF16 = ml_dtypes.bfloat16

_CACHE = {}


def _token_constants(tokens, W1, b1, W2, b2, Wa, ba):
    """Fold the graph-independent prompt-token stream (all f32 numpy)."""
    t = tokens.astype(np.float32)

    def sigmoid(v):
        return (1.0 / (1.0 + np.exp(-v.astype(np.float32)))).astype(np.float32)

    M_in = (sigmoid(t @ t.T) >= INNER_PRUNE).astype(np.float32)
    deg_tok = 1.0 + M_in.sum(0)
    inv_tok = (1.0 / np.sqrt(deg_tok)).astype(np.float32)
    norm_in = M_in * inv_tok[:, None] * inv_tok[None, :]
    ht1lin = t @ W1
    out_tok1 = norm_in @ ht1lin + ht1lin * (1.0 / deg_tok)[:, None] + b1
    ht1a = np.where(out_tok1 >= 0, out_tok1, NEG_SLOPE * out_tok1).astype(np.float32)
    ht2lin = ht1a @ W2
    out_tok2 = norm_in @ ht2lin + ht2lin * (1.0 / deg_tok)[:, None] + b2
    tok_sum2 = out_tok2.sum(0).astype(np.float32)
    cT1 = inv_tok[:, None] * ht1lin
    cT2 = inv_tok[:, None] * ht2lin
    cT12 = np.concatenate([cT1, cT2], axis=1).astype(np.float32)  # [10, 128]
    return cT12, tok_sum2


def _host_graph_prep(src, dst):
    """Dense count matrix A[src, dst] (bf16, tile-grid layout) + in-degrees."""
    src = src.astype(np.int64)
    dst = dst.astype(np.int64)
    cnt = np.bincount(src * N + dst, minlength=N * N).reshape(N, N)
    # A_w[p, s*1024 + d] = #edges (s*128+p) -> d
    A_w = np.ascontiguousarray(
        cnt.reshape(8, 128, N).transpose(1, 0, 2).reshape(128, 8 * N)
    ).astype(BF16)
    indeg = np.bincount(dst, minlength=N).astype(np.float32)
    indeg_w = np.ascontiguousarray(indeg.reshape(8, 128).T)  # [p, t] = indeg[t*128+p]
    return A_w, indeg_w


def _build_program(bloc):
    from concourse import bacc, tile, mybir

    fp32 = mybir.dt.float32
    bf16 = mybir.dt.bfloat16
    AF = mybir.ActivationFunctionType
    ALU = mybir.AluOpType

    nc = bacc.Bacc("TRN2", target_bir_lowering=False, debug=True)

    x_p = nc.declare_dram_parameter("x", [bloc, N, F], fp32, isOutput=False)
    A_p = nc.declare_dram_parameter("A", [bloc, 128, 8 * N], bf16, isOutput=False)
    indeg_p = nc.declare_dram_parameter("indeg", [bloc, 128, 8], fp32, isOutput=False)
    W1_p = nc.declare_dram_parameter("W1", [F, H], fp32, isOutput=False)
    tokT_p = nc.declare_dram_parameter("tokT", [F, T], fp32, isOutput=False)
    cT12_p = nc.declare_dram_parameter("cT12", [T, 2 * H], fp32, isOutput=False)
    b1t8_p = nc.declare_dram_parameter("b1t8", [128, 8 * H], fp32, isOutput=False)
    W2_p = nc.declare_dram_parameter("W2", [H, H], fp32, isOutput=False)
    const64_p = nc.declare_dram_parameter("c64", [H, 1], fp32, isOutput=False)
    Wa_p = nc.declare_dram_parameter("Wa", [H, C], fp32, isOutput=False)
    bat_p = nc.declare_dram_parameter("bat", [bloc, C], fp32, isOutput=False)
    ident_p = nc.declare_dram_parameter("ident", [128, 128], fp32, isOutput=False)
    out_p = nc.declare_dram_parameter("out", [bloc, C], fp32, isOutput=True)

    with tile.TileContext(nc) as tc:
        with (
            tc.tile_pool(name="const", bufs=1) as cpool,
            tc.tile_pool(name="adj", bufs=2) as apool,
            tc.tile_pool(name="xp", bufs=2) as xpool,
            tc.tile_pool(name="work", bufs=2) as wpool,
            tc.tile_pool(name="ps", bufs=3, space="PSUM") as pspool,
            tc.tile_pool(name="psm", bufs=1, space="PSUM") as psmpool,
            tc.tile_pool(name="psb", bufs=1, space="PSUM") as psbpool,
        ):
            # ---- constants ----
            W1_t = cpool.tile([F, H], fp32)
            nc.sync.dma_start(out=W1_t[:], in_=W1_p[:])
            tokT_t = cpool.tile([F, T], fp32)
            nc.sync.dma_start(out=tokT_t[:], in_=tokT_p[:])
            cT12_t = cpool.tile([T, 2 * H], fp32)
            nc.sync.dma_start(out=cT12_t[:], in_=cT12_p[:])
            cT12_b = cpool.tile([T, 2 * H], bf16)
            nc.vector.tensor_copy(cT12_b[:], cT12_t[:])
            b1t8_t = cpool.tile([128, 8 * H], fp32)
            nc.sync.dma_start(out=b1t8_t[:], in_=b1t8_p[:])
            W2_t = cpool.tile([H, H], fp32)
            nc.sync.dma_start(out=W2_t[:], in_=W2_p[:])
            c64_t = cpool.tile([H, 1], fp32)
            nc.sync.dma_start(out=c64_t[:], in_=const64_p[:])
            Wa_t = cpool.tile([H, C], fp32)
            nc.sync.dma_start(out=Wa_t[:], in_=Wa_p[:])
            bat_t = cpool.tile([bloc, C], fp32)
            nc.sync.dma_start(out=bat_t[:], in_=bat_p[:])
            ident_t = cpool.tile([128, 128], fp32)
            nc.sync.dma_start(out=ident_t[:], in_=ident_p[:])
            ones10 = cpool.tile([T, 1], bf16)
            nc.vector.memset(ones10[:], 1.0)

            # head accumulators
            SD_T = cpool.tile([H, bloc], fp32)
            Z_T = cpool.tile([H, bloc], fp32)

            for g in range(bloc):
                # ---- loads ----
                A_t = apool.tile([128, 8 * N], bf16, tag="A")
                nc.sync.dma_start(out=A_t[:], in_=A_p[g])
                x_t = xpool.tile([128, 8, F], fp32, tag="x")
                nc.sync.dma_start(out=x_t[:], in_=x_p[g].rearrange("(t p) f -> p t f", p=128))
                indeg_t = wpool.tile([128, 8], fp32, tag="indeg")
                nc.sync.dma_start(out=indeg_t[:], in_=indeg_p[g])

                # ---- transpose x -> xT [128f, 8*128n] ----
                xT = xpool.tile([F, 8, 128], fp32, tag="xT")
                for t in range(8):
                    ps = pspool.tile([128, 128], fp32, tag="work")
                    nc.tensor.transpose(ps[:], x_t[:, t, :], ident_t[:])
                    nc.scalar.copy(xT[:, t, :], ps[:])

                # ---- M_cr mask [10, 1024] bf16 ----
                mcr_ps = psmpool.tile([T, N], fp32, tag="mcr")
                for hblk in range(2):
                    nc.tensor.matmul(
                        mcr_ps[:, hblk * 512:(hblk + 1) * 512],
                        tokT_t[:],
                        xT[:].rearrange("p t n -> p (t n)")[:, hblk * 512:(hblk + 1) * 512],
                        start=True, stop=True)
                mask_b = wpool.tile([T, N], bf16, tag="mask")
                nc.vector.tensor_scalar(mask_b[:], mcr_ps[:], THR_CROSS, None, ALU.is_ge)

                # ---- M_cr column sums -> [128, 8] (per node) ----
                mcrcol_ps = psbpool.tile([128, 8], fp32, tag="misc")
                for t in range(8):
                    nc.tensor.matmul(mcrcol_ps[:, t:t + 1],
                                     mask_b[:, t * 128:(t + 1) * 128],
                                     ones10[:], start=True, stop=True)

                # ---- degrees / normalization ----
                dn = wpool.tile([128, 8], fp32, tag="dn")
                nc.vector.tensor_tensor(dn[:], indeg_t[:], mcrcol_ps[:], ALU.add)
                nc.vector.tensor_scalar_add(dn[:], dn[:], 1.0)
                inv2c = wpool.tile([128, 8], fp32, tag="inv2c")
                nc.vector.reciprocal(inv2c[:], dn[:])
                invc = wpool.tile([128, 8], fp32, tag="invc")
                nc.scalar.sqrt(invc[:], inv2c[:])

                # ---- h1 = inv * (x @ W1) ----
                h1 = xpool.tile([128, 8, H], fp32, tag="h1")
                for t in range(8):
                    hps = pspool.tile([128, H], fp32, tag="work")
                    nc.tensor.matmul(hps[:], xT[:, t, :], W1_t[:], start=True, stop=True)
                    nc.scalar.activation(h1[:, t, :], hps[:], AF.Copy,
                                         scale=invc[:, t:t + 1])
                h1b = xpool.tile([128, 8, H], bf16, tag="h1b")
                nc.vector.tensor_copy(h1b[:], h1[:])

                # ---- cross terms [128n, 128] per tile ----
                crs = xpool.tile([128, 8, 2 * H], fp32, tag="crs")
                for t in range(8):
                    cps = pspool.tile([128, 2 * H], fp32, tag="work")
                    nc.tensor.matmul(cps[:], mask_b[:, t * 128:(t + 1) * 128],
                                     cT12_b[:], start=True, stop=True)
                    nc.scalar.copy(crs[:, t, :], cps[:])

                # ---- layer 1 aggregation: y = A^T @ h1 (8x8 grid) ----
                c1h = xpool.tile([128, 8, H], fp32, tag="c1h")
                nc.vector.tensor_tensor(c1h[:], crs[:, :, 0:H], h1[:], ALU.add)
                hn = xpool.tile([128, 8, H], fp32, tag="hn")
                for t in range(8):
                    yps = pspool.tile([128, H], fp32, tag="work")
                    for s in range(8):
                        nc.tensor.matmul(yps[:],
                                         A_t[:, s * N + t * 128: s * N + (t + 1) * 128],
                                         h1b[:, s, :],
                                         start=(s == 0), stop=(s == 7))
                    nc.vector.tensor_tensor(hn[:, t, :], yps[:], c1h[:, t, :], ALU.add)
                    nc.scalar.activation(hn[:, t, :], hn[:, t, :], AF.Copy,
                                         scale=invc[:, t:t + 1])

                # ---- hn1a = lrelu(hn + b1) ----
                nc.vector.tensor_tensor(hn[:], hn[:],
                                        b1t8_t[:].rearrange("p (t f) -> p t f", t=8),
                                        ALU.add)
                hnm = xpool.tile([128, 8, H], fp32, tag="hnm")
                nc.vector.tensor_scalar_mul(hnm[:], hn[:], NEG_SLOPE)
                nc.vector.tensor_tensor(hn[:], hn[:], hnm[:], ALU.max)

                # ---- dvec, zvec accumulation; g2 = inv * hn1a ----
                dv_ps = psbpool.tile([H, 1], fp32, tag="dv")
                zv_ps = psbpool.tile([H, 1], fp32, tag="zv")
                g2 = xpool.tile([128, 8, H], fp32, tag="g2")
                for t in range(8):
                    nc.tensor.matmul(dv_ps[:], hn[:, t, :], inv2c[:, t:t + 1],
                                     start=(t == 0), stop=(t == 7))
                    nc.tensor.matmul(zv_ps[:], crs[:, t, H:2 * H], invc[:, t:t + 1],
                                     start=(t == 0), stop=(t == 7))
                    nc.scalar.activation(g2[:, t, :], hn[:, t, :], AF.Copy,
                                         scale=invc[:, t:t + 1])
                g2b = xpool.tile([128, 8, H], bf16, tag="g2b")
                nc.vector.tensor_copy(g2b[:], g2[:])

                # ---- layer 2 aggregation + cvec ----
                agg2 = xpool.tile([128, 8, H], fp32, tag="agg2")
                for t in range(8):
                    aps = pspool.tile([128, H], fp32, tag="work")
                    for s in range(8):
                        nc.tensor.matmul(aps[:],
                                         A_t[:, s * N + t * 128: s * N + (t + 1) * 128],
                                         g2b[:, s, :],
                                         start=(s == 0), stop=(s == 7))
                    nc.scalar.copy(agg2[:, t, :], aps[:])
                cv_ps = psbpool.tile([H, 1], fp32, tag="misc")
                for t in range(8):
                    nc.tensor.matmul(cv_ps[:], agg2[:, t, :], invc[:, t:t + 1],
                                     start=(t == 0), stop=(t == 7))

                # ---- per-graph head columns ----
                dvs = wpool.tile([H, 1], fp32, tag="dvs")
                nc.scalar.copy(dvs[:], dv_ps[:])
                nc.vector.tensor_tensor(SD_T[:, g:g + 1], cv_ps[:], dvs[:], ALU.add)
                nc.scalar.copy(Z_T[:, g:g + 1], zv_ps[:])

            # ---- batched head ----
            emb_ps = psbpool.tile([H, bloc], fp32, tag="misc")
            nc.tensor.matmul(emb_ps[:], W2_t[:], SD_T[:], start=True, stop=True)
            embT = cpool.tile([H, bloc], fp32)
            nc.vector.tensor_tensor(embT[:], emb_ps[:], Z_T[:], ALU.add)
            nc.vector.tensor_scalar(embT[:], embT[:], c64_t[:], None, ALU.add)
            lg_ps = psbpool.tile([bloc, C], fp32, tag="misc")
            nc.tensor.matmul(lg_ps[:], embT[:], Wa_t[:], start=True, stop=True)
            lg = cpool.tile([bloc, C], fp32)
            nc.vector.tensor_tensor(lg[:], lg_ps[:], bat_t[:], ALU.add)
            mx = cpool.tile([bloc, 1], fp32)
            nc.vector.tensor_reduce(mx[:], lg[:], mybir.AxisListType.X, ALU.max)
            nmx = cpool.tile([bloc, 1], fp32)
            nc.vector.tensor_scalar_mul(nmx[:], mx[:], -1.0)
            ex = cpool.tile([bloc, C], fp32)
            nc.scalar.activation(ex[:], lg[:], AF.Exp, bias=nmx[:])
            sm = cpool.tile([bloc, 1], fp32)
            nc.vector.tensor_reduce(sm[:], ex[:], mybir.AxisListType.X, ALU.add)
            rs = cpool.tile([bloc, 1], fp32)
            nc.vector.reciprocal(rs[:], sm[:])
            outt = cpool.tile([bloc, C], fp32)
            nc.vector.tensor_scalar(outt[:], ex[:], rs[:], None, ALU.mult)
            nc.sync.dma_start(out=out_p[:], in_=outt[:])

    nc.compile()
    return nc


def _get_program(bloc=BLOC):
    if bloc not in _CACHE:
        _CACHE[bloc] = _build_program(bloc)
    return _CACHE[bloc]


def build_in_maps(x, tokens, W1, b1, W2, b2, Wa, ba, edge_src, edge_dst,
                  ncores=NCORES, bloc=BLOC):
    x = np.asarray(x, np.float32)
    cT12, tok_sum2 = _token_constants(
        np.asarray(tokens, np.float32), np.asarray(W1, np.float32),
        np.asarray(b1, np.float32), np.asarray(W2, np.float32),
        np.asarray(b2, np.float32), np.asarray(Wa, np.float32),
        np.asarray(ba, np.float32))
    const64 = (N * np.asarray(b2, np.float32) + tok_sum2).reshape(H, 1)
    shared = {
        "W1": np.asarray(W1, np.float32),
        "tokT": np.ascontiguousarray(np.asarray(tokens, np.float32).T),
        "cT12": cT12,
        "b1t8": np.tile(np.asarray(b1, np.float32)[None, :], (128, 8)),
        "W2": np.asarray(W2, np.float32),
        "c64": const64,
        "Wa": (np.asarray(Wa, np.float32) / float(T + N)),
        "bat": np.tile(np.asarray(ba, np.float32)[None, :], (bloc, 1)),
        "ident": np.eye(128, dtype=np.float32),
    }
    in_maps = []
    for c in range(ncores):
        A_w = np.zeros((bloc, 128, 8 * N), BF16)
        indeg_w = np.zeros((bloc, 128, 8), np.float32)
        for g in range(bloc):
            gi = c * bloc + g
            A_w[g], indeg_w[g] = _host_graph_prep(
                np.asarray(edge_src[gi]), np.asarray(edge_dst[gi]))
        m = dict(shared)
        m["x"] = np.ascontiguousarray(x[c * bloc:(c + 1) * bloc])
        m["A"] = A_w
        m["indeg"] = indeg_w
        in_maps.append(m)
    return in_maps


def kernel(x, tokens, W1, b1, W2, b2, Wa, ba, edge_src, edge_dst):
    from concourse.bass_utils import run_bass_kernel_spmd

    nc = _get_program()
    in_maps = build_in_maps(x, tokens, W1, b1, W2, b2, Wa, ba, edge_src, edge_dst)
    res = run_bass_kernel_spmd(nc, in_maps, list(range(NCORES)))
    out = np.concatenate([res.results[c]["out"] for c in range(NCORES)], axis=0)
    return out.astype(np.float32)


# revision 11
# speedup vs baseline: 9.8495x; 1.0539x over previous
"""Trainium2 Bass kernel for the prompted-GCN pipeline (gnn_message_passing).

Data-parallel over the graph batch: 8 NeuronCores x 8 graphs each.

Sharding/layout choice (host side, per the free-choice sharding contract):
the host re-encodes each graph's edge list as a dense count matrix
Ahat[src, dst] = #edges(src->dst) + I (self-loop folded in), packed fp8 in
DoubleRow pair layout, and folds the graph-independent prompt-token stream
into constants. All x/edge VALUE computation (matmuls, masks, degrees,
normalization, aggregation, pooling, softmax) runs on device.

Device algorithm per graph (H-major feature layout, no gathers):
  Z = tokens @ xT; M_cr = (Z >= logit(0.1))        [fp32r matmul]
  deg = 1 + indeg + colsum(M_cr); inv = rsqrt(deg) [node-major]
  invrep[64,1024] = ones64 (x) inv                 [rank-1 matmuls]
  h1 = fp8(inv * (x @ W1))                         [node-major, L1 operand]
  yT = h1^T @ Ahat   (fp8 DoubleRow, out [H, N])   [self-term inside Ahat]
  crsT = cT12^T @ M_cr                             [one bf16 matmul]
  hnT = lrelu((yT + crs1T) * invrep + b1)          [scalar Lrelu fused]
  g2T8 = fp8(hnT * invrep); g2 node-major via PE transposes
  agg2T = g2^T @ Ahat  (fp8 DoubleRow)
  sd = sum_n ((agg2T + crs2T) * invrep)[:, n]      [g2 self-term in Ahat]
  out = softmax((sd @ W2 + N*b2 + tok_sum2) @ Wa/(T+N) + ba)
"""

import sys

sys.path.insert(0, '/opt/trn_rl_repo')
import antenv  # noqa: E402

if '/opt/trn_rl_repo/antenv' not in antenv.__path__:
    antenv.__path__.append('/opt/trn_rl_repo/antenv')

import numpy as np  # noqa: E402
import ml_dtypes  # noqa: E402

B, N, E, F, H, T, C = 64, 1024, 16384, 128, 64, 10, 2
NCORES = 8
BLOC = B // NCORES
NEG_SLOPE = 0.01
INNER_PRUNE, CROSS_PRUNE = 0.3, 0.1
THR_CROSS = float(np.log(CROSS_PRUNE / (1.0 - CROSS_PRUNE)))  # sigmoid(z)>=p  <=>  z>=logit(p)
FP8 = ml_dtypes.float8_e4m3

_CACHE = {}


def _token_constants(tokens, W1, b1, W2, b2, Wa, ba):
    """Fold the graph-independent prompt-token stream (all f32 numpy)."""
    t = tokens.astype(np.float32)

    def sigmoid(v):
        return (1.0 / (1.0 + np.exp(-v.astype(np.float32)))).astype(np.float32)

    M_in = (sigmoid(t @ t.T) >= INNER_PRUNE).astype(np.float32)
    deg_tok = 1.0 + M_in.sum(0)
    inv_tok = (1.0 / np.sqrt(deg_tok)).astype(np.float32)
    norm_in = M_in * inv_tok[:, None] * inv_tok[None, :]
    ht1lin = t @ W1
    out_tok1 = norm_in @ ht1lin + ht1lin * (1.0 / deg_tok)[:, None] + b1
    ht1a = np.where(out_tok1 >= 0, out_tok1, NEG_SLOPE * out_tok1).astype(np.float32)
    ht2lin = ht1a @ W2
    out_tok2 = norm_in @ ht2lin + ht2lin * (1.0 / deg_tok)[:, None] + b2
    tok_sum2 = out_tok2.sum(0).astype(np.float32)
    cT1 = inv_tok[:, None] * ht1lin
    cT2p = inv_tok[:, None] * ht1a          # W2 deferred to the head
    cT12 = np.concatenate([cT1, cT2p], axis=1).astype(np.float32)  # [10, 128]
    return cT12, tok_sum2


def _host_graph_prep(src, dst):
    """Ahat = count(src->dst) + I in fp8 DoubleRow pair layout + in-degrees."""
    src = src.astype(np.int64)
    dst = dst.astype(np.int64)
    cnt = np.bincount(src * N + dst, minlength=N * N).reshape(N, N)
    cnt = cnt.astype(np.float32)
    cnt[np.arange(N), np.arange(N)] += 1.0      # fold self-loop term
    # A8[p, u, i, d] = Ahat[(2u+i)*128+p, d]
    A8 = np.ascontiguousarray(
        cnt.reshape(4, 2, 128, N).transpose(2, 0, 1, 3)
    ).astype(FP8)
    indeg = np.bincount(dst, minlength=N).astype(np.float32)
    indeg_w = np.ascontiguousarray(indeg.reshape(8, 128).T)  # [p, t] = indeg[t*128+p]
    return A8, indeg_w


def _build_program(bloc):
    from concourse import bacc, tile, mybir

    fp32 = mybir.dt.float32
    fp32r = mybir.dt.float32r
    bf16 = mybir.dt.bfloat16
    fp8 = mybir.dt.float8e4
    AF = mybir.ActivationFunctionType
    ALU = mybir.AluOpType
    DR = mybir.MatmulPerfMode.DoubleRow

    nc = bacc.Bacc("TRN2", target_bir_lowering=False, debug=True)

    xT_p = nc.declare_dram_parameter("xT", [bloc, F, N], fp32r, isOutput=False)
    A8_p = nc.declare_dram_parameter("A8", [bloc, 128, 4, 2, N], fp8, isOutput=False)
    indeg_p = nc.declare_dram_parameter("indeg", [bloc, 128, 8], fp32, isOutput=False)
    W1_p = nc.declare_dram_parameter("W1", [F, H], fp32r, isOutput=False)
    tokT_p = nc.declare_dram_parameter("tokT", [F, T], fp32r, isOutput=False)
    cT12_p = nc.declare_dram_parameter("cT12", [T, 2 * H], fp32, isOutput=False)
    b1c_p = nc.declare_dram_parameter("b1c", [H, 1], fp32, isOutput=False)
    W2_p = nc.declare_dram_parameter("W2", [H, H], fp32, isOutput=False)
    const64_p = nc.declare_dram_parameter("c64", [H, 1], fp32, isOutput=False)
    Wa_p = nc.declare_dram_parameter("Wa", [H, C], fp32, isOutput=False)
    bat_p = nc.declare_dram_parameter("bat", [bloc, C], fp32, isOutput=False)
    ident_p = nc.declare_dram_parameter("ident", [128, 128], fp32, isOutput=False)
    idb_p = nc.declare_dram_parameter("idb", [128, 128], bf16, isOutput=False)
    out_p = nc.declare_dram_parameter("out", [bloc, C], fp32, isOutput=True)
    dinv = [nc.dram_tensor(f"dinv{g}", [N], fp32) for g in range(bloc)]

    with tile.TileContext(nc) as tc:
        with (
            tc.tile_pool(name="const", bufs=1) as cpool,
            tc.tile_pool(name="adj", bufs=2) as apool,
            tc.tile_pool(name="xp", bufs=2) as xpool,
            tc.tile_pool(name="work", bufs=2) as wpool,
            tc.tile_pool(name="psw", bufs=2, space="PSUM") as pswork,
            tc.tile_pool(name="psbig", bufs=1, space="PSUM") as psbig,
            tc.tile_pool(name="psagg", bufs=2, space="PSUM") as psagg,
        ):
            # ---- constants ----
            W1_t = cpool.tile([F, H], fp32r)
            nc.sync.dma_start(out=W1_t[:], in_=W1_p[:])
            tokT_t = cpool.tile([F, T], fp32r)
            nc.sync.dma_start(out=tokT_t[:], in_=tokT_p[:])
            cT12_t = cpool.tile([T, 2 * H], fp32)
            nc.sync.dma_start(out=cT12_t[:], in_=cT12_p[:])
            cT12_b = cpool.tile([T, 2 * H], bf16)
            nc.vector.tensor_copy(cT12_b[:], cT12_t[:])
            b1c_t = cpool.tile([H, 1], fp32)
            nc.sync.dma_start(out=b1c_t[:], in_=b1c_p[:])
            W2_t = cpool.tile([H, H], fp32)
            nc.sync.dma_start(out=W2_t[:], in_=W2_p[:])
            c64_t = cpool.tile([H, 1], fp32)
            nc.sync.dma_start(out=c64_t[:], in_=const64_p[:])
            Wa_t = cpool.tile([H, C], fp32)
            nc.sync.dma_start(out=Wa_t[:], in_=Wa_p[:])
            bat_t = cpool.tile([bloc, C], fp32)
            nc.sync.dma_start(out=bat_t[:], in_=bat_p[:])
            ident_t = cpool.tile([128, 128], fp32)
            nc.sync.dma_start(out=ident_t[:], in_=ident_p[:])
            idb_t = cpool.tile([128, 128], bf16)
            nc.sync.dma_start(out=idb_t[:], in_=idb_p[:])
            ones10 = cpool.tile([T, 1], bf16)
            nc.vector.memset(ones10[:], 1.0)
            ones64r = cpool.tile([1, H], fp32)
            nc.vector.memset(ones64r[:], 1.0)

            SD_T = cpool.tile([H, bloc], fp32)

            for g in range(bloc):
                # ---- loads ----
                xT = xpool.tile([F, N], fp32r, tag="xT")
                nc.sync.dma_start(out=xT[:], in_=xT_p[g])
                A8_t = apool.tile([128, 4, 2, N], fp8, tag="A")
                nc.sync.dma_start(out=A8_t[:], in_=A8_p[g])
                indeg_t = wpool.tile([128, 8], fp32, tag="indeg")
                nc.sync.dma_start(out=indeg_t[:], in_=indeg_p[g])

                # ---- M_cr mask [10, 1024] (fp32r matmul) ----
                mcr_ps = psbig.tile([T, N], fp32, tag="big")
                for hblk in range(2):
                    nc.tensor.matmul(
                        mcr_ps[:, hblk * 512:(hblk + 1) * 512],
                        tokT_t[:],
                        xT[:, hblk * 512:(hblk + 1) * 512],
                        start=True, stop=True)
                mask_b = wpool.tile([T, N], bf16, tag="mask")
                nc.vector.tensor_scalar(mask_b[:], mcr_ps[:], THR_CROSS, None, ALU.is_ge)

                # ---- M_cr column sums -> [128, 8] (per node) ----
                mcrcol_ps = pswork.tile([128, 8], fp32, tag="work")
                for t in range(8):
                    nc.tensor.matmul(mcrcol_ps[:, t:t + 1],
                                     mask_b[:, t * 128:(t + 1) * 128],
                                     ones10[:], start=(t == 0), stop=(t == 7))

                # ---- degrees / normalization (node-major) ----
                dn = wpool.tile([128, 8], fp32, tag="dn")
                nc.vector.tensor_tensor(dn[:], indeg_t[:], mcrcol_ps[:], ALU.add)
                nc.vector.tensor_scalar_add(dn[:], dn[:], 1.0)
                inv2c = wpool.tile([128, 8], fp32, tag="inv2c")
                nc.vector.reciprocal(inv2c[:], dn[:])
                invc = wpool.tile([128, 8], fp32, tag="invc")
                nc.scalar.sqrt(invc[:], inv2c[:])

                # ---- invrep [64, 1024]: DRAM round-trip + partition broadcast ----
                nc.sync.dma_start(out=dinv[g].rearrange("(t p) -> p t", p=128),
                                  in_=invc[:])
                invrow = wpool.tile([1, N], fp32, tag="invrow")
                nc.sync.dma_start(out=invrow[:],
                                  in_=dinv[g].rearrange("(o n) -> o n", o=1))
                invrep = xpool.tile([H, N], fp32, tag="invrep")
                nc.gpsimd.partition_broadcast(invrep[:], invrow[:])

                # ---- cross terms crsT [128=2H, 1024] -> SBUF ----
                crsT_ps = psbig.tile([2 * H, N], fp32, tag="big")
                for hblk in range(2):
                    nc.tensor.matmul(crsT_ps[:, hblk * 512:(hblk + 1) * 512],
                                     cT12_b[:],
                                     mask_b[:, hblk * 512:(hblk + 1) * 512],
                                     start=True, stop=True)
                crsT = xpool.tile([2 * H, N], fp32, tag="crsT")
                nc.scalar.copy(crsT[:], crsT_ps[:])

                # ---- h1 node-major fp8 = fp8(inv * (x @ W1)) ----
                h1b = xpool.tile([128, 8, H], fp8, tag="h1b")
                for t in range(8):
                    hps = pswork.tile([128, H], fp32, tag="work")
                    nc.tensor.matmul(hps[:], xT[:, t * 128:(t + 1) * 128], W1_t[:],
                                     start=True, stop=True)
                    nc.scalar.activation(h1b[:, t, :], hps[:], AF.Copy,
                                         scale=invc[:, t:t + 1])

                # ---- layer 1: yT = h1^T @ Ahat (fp8 DoubleRow) ----
                yT_ps = psagg.tile([H, N], fp32, tag="agg")
                for hf in range(2):
                    for u in range(4):
                        nc.tensor.matmul(
                            yT_ps[:, hf * 512:(hf + 1) * 512],
                            h1b[:, 2 * u:2 * u + 2, :],
                            A8_t[:, u, :, hf * 512:(hf + 1) * 512],
                            start=(u == 0), stop=(u == 3), perf_mode=DR)

                # ---- hnT = lrelu((yT + crs1T) * invrep + b1) ----
                tmp = xpool.tile([H, N], fp32, tag="tmp")
                nc.vector.tensor_tensor(tmp[:], yT_ps[:], crsT[0:H, :], ALU.add)
                nc.vector.tensor_tensor(tmp[:], tmp[:], invrep[:], ALU.mult)
                hnT = xpool.tile([H, N], fp32, tag="hnT")
                nc.scalar.activation(hnT[:], tmp[:], AF.Lrelu,
                                     bias=b1c_t[:], alpha=NEG_SLOPE)

                # ---- g2 = fp8(inv * hn), node-major via PE transposes ----
                g2Tb = xpool.tile([H, N], bf16, tag="g2Tb")
                nc.vector.tensor_tensor(g2Tb[:], hnT[:], invrep[:], ALU.mult)
                g2b = xpool.tile([128, 8, H], fp8, tag="g2b")
                for t in range(8):
                    tps = pswork.tile([128, H], bf16, tag="work")
                    nc.tensor.transpose(tps[:], g2Tb[:, t * 128:(t + 1) * 128],
                                        idb_t[0:H, 0:H])
                    nc.scalar.copy(g2b[:, t, :], tps[:])

                # ---- layer 2: agg2T = g2^T @ Ahat (fp8 DoubleRow) ----
                a2_ps = psagg.tile([H, N], fp32, tag="agg")
                for hf in range(2):
                    for u in range(4):
                        nc.tensor.matmul(
                            a2_ps[:, hf * 512:(hf + 1) * 512],
                            g2b[:, 2 * u:2 * u + 2, :],
                            A8_t[:, u, :, hf * 512:(hf + 1) * 512],
                            start=(u == 0), stop=(u == 3), perf_mode=DR)

                # ---- sd = sum_n ((agg2T + crs2T) * invrep)[:, n] ----
                nc.vector.tensor_tensor(tmp[:], a2_ps[:], crsT[H:2 * H, :], ALU.add)
                nc.vector.tensor_tensor(tmp[:], tmp[:], invrep[:], ALU.mult)
                nc.vector.tensor_reduce(SD_T[:, g:g + 1], tmp[:],
                                        mybir.AxisListType.X, ALU.add)

            # ---- batched head ----
            emb_ps = pswork.tile([H, bloc], fp32, tag="work")
            nc.tensor.matmul(emb_ps[:], W2_t[:], SD_T[:], start=True, stop=True)
            embT = cpool.tile([H, bloc], fp32)
            nc.vector.tensor_scalar(embT[:], emb_ps[:], c64_t[:], None, ALU.add)
            lg_ps = pswork.tile([bloc, C], fp32, tag="work")
            nc.tensor.matmul(lg_ps[:], embT[:], Wa_t[:], start=True, stop=True)
            lg = cpool.tile([bloc, C], fp32)
            nc.vector.tensor_tensor(lg[:], lg_ps[:], bat_t[:], ALU.add)
            mx = cpool.tile([bloc, 1], fp32)
            nc.vector.tensor_reduce(mx[:], lg[:], mybir.AxisListType.X, ALU.max)
            nmx = cpool.tile([bloc, 1], fp32)
            nc.vector.tensor_scalar_mul(nmx[:], mx[:], -1.0)
            ex = cpool.tile([bloc, C], fp32)
            nc.scalar.activation(ex[:], lg[:], AF.Exp, bias=nmx[:])
            sm = cpool.tile([bloc, 1], fp32)
            nc.vector.tensor_reduce(sm[:], ex[:], mybir.AxisListType.X, ALU.add)
            rs = cpool.tile([bloc, 1], fp32)
            nc.vector.reciprocal(rs[:], sm[:])
            outt = cpool.tile([bloc, C], fp32)
            nc.vector.tensor_scalar(outt[:], ex[:], rs[:], None, ALU.mult)
            nc.sync.dma_start(out=out_p[:], in_=outt[:])

    nc.compile()
    return nc


def _get_program(bloc=BLOC):
    if bloc not in _CACHE:
        _CACHE[bloc] = _build_program(bloc)
    return _CACHE[bloc]


def build_in_maps(x, tokens, W1, b1, W2, b2, Wa, ba, edge_src, edge_dst,
                  ncores=NCORES, bloc=BLOC):
    x = np.asarray(x, np.float32)
    cT12, tok_sum2 = _token_constants(
        np.asarray(tokens, np.float32), np.asarray(W1, np.float32),
        np.asarray(b1, np.float32), np.asarray(W2, np.float32),
        np.asarray(b2, np.float32), np.asarray(Wa, np.float32),
        np.asarray(ba, np.float32))
    const64 = (N * np.asarray(b2, np.float32) + tok_sum2).reshape(H, 1)
    shared = {
        "W1": np.asarray(W1, np.float32),
        "tokT": np.ascontiguousarray(np.asarray(tokens, np.float32).T),
        "cT12": cT12,
        "b1c": np.asarray(b1, np.float32).reshape(H, 1),
        "W2": np.asarray(W2, np.float32),
        "c64": const64,
        "Wa": (np.asarray(Wa, np.float32) / float(T + N)),
        "bat": np.tile(np.asarray(ba, np.float32)[None, :], (bloc, 1)),
        "ident": np.eye(128, dtype=np.float32),
        "idb": np.eye(128, dtype=np.float32).astype(ml_dtypes.bfloat16),
    }
    in_maps = []
    for c in range(ncores):
        A8 = np.zeros((bloc, 128, 4, 2, N), FP8)
        indeg_w = np.zeros((bloc, 128, 8), np.float32)
        xTl = np.zeros((bloc, F, N), np.float32)
        for g in range(bloc):
            gi = c * bloc + g
            A8[g], indeg_w[g] = _host_graph_prep(
                np.asarray(edge_src[gi]), np.asarray(edge_dst[gi]))
            xTl[g] = x[gi].T
        m = dict(shared)
        m["xT"] = xTl
        m["A8"] = A8
        m["indeg"] = indeg_w
        in_maps.append(m)
    return in_maps


def kernel(x, tokens, W1, b1, W2, b2, Wa, ba, edge_src, edge_dst):
    from concourse.bass_utils import run_bass_kernel_spmd

    nc = _get_program()
    in_maps = build_in_maps(x, tokens, W1, b1, W2, b2, Wa, ba, edge_src, edge_dst)
    res = run_bass_kernel_spmd(nc, in_maps, list(range(NCORES)))
    out = np.concatenate([res.results[c]["out"] for c in range(NCORES)], axis=0)
    return out.astype(np.float32)


# revision 16
# speedup vs baseline: 12.0957x; 1.2281x over previous
"""Trainium2 Bass kernel for the prompted-GCN pipeline (gnn_message_passing).

Data-parallel over the graph batch: 8 NeuronCores x 8 graphs each.

Sharding/layout choice (host side, per the free-choice sharding contract):
the host re-encodes each graph's edge list as a dense count matrix
Ahat[src, dst] = #edges(src->dst) + I (self-loop folded in), packed fp8 in
DoubleRow pair layout, and folds the graph-independent prompt-token stream
into constants. All x/edge VALUE computation (matmuls, masks, degrees,
normalization, aggregation, pooling, softmax) runs on device.

Device algorithm per graph (H-major feature layout, no gathers):
  Z = tokens @ xT; M_cr = (Z >= logit(0.1))        [fp32r matmul]
  deg = 1 + indeg + colsum(M_cr); inv = rsqrt(deg) [node-major]
  invrep[64,1024] = ones64 (x) inv                 [rank-1 matmuls]
  h1 = fp8(inv * (x @ W1))                         [node-major, L1 operand]
  yT = h1^T @ Ahat   (fp8 DoubleRow, out [H, N])   [self-term inside Ahat]
  crsT = cT12^T @ M_cr                             [one bf16 matmul]
  hnT = lrelu((yT + crs1T) * invrep + b1)          [scalar Lrelu fused]
  g2T8 = fp8(hnT * invrep); g2 node-major via PE transposes
  agg2T = g2^T @ Ahat  (fp8 DoubleRow)
  sd = sum_n ((agg2T + crs2T) * invrep)[:, n]      [g2 self-term in Ahat]
  out = softmax((sd @ W2 + N*b2 + tok_sum2) @ Wa/(T+N) + ba)
"""

import sys

sys.path.insert(0, '/opt/trn_rl_repo')
import antenv  # noqa: E402

if '/opt/trn_rl_repo/antenv' not in antenv.__path__:
    antenv.__path__.append('/opt/trn_rl_repo/antenv')

import numpy as np  # noqa: E402
import ml_dtypes  # noqa: E402

B, N, E, F, H, T, C = 64, 1024, 16384, 128, 64, 10, 2
NCORES = 8
BLOC = B // NCORES
NEG_SLOPE = 0.01
INNER_PRUNE, CROSS_PRUNE = 0.3, 0.1
THR_CROSS = float(np.log(CROSS_PRUNE / (1.0 - CROSS_PRUNE)))  # sigmoid(z)>=p  <=>  z>=logit(p)
FP8 = ml_dtypes.float8_e4m3

_CACHE = {}


def _token_constants(tokens, W1, b1, W2, b2, Wa, ba):
    """Fold the graph-independent prompt-token stream (all f32 numpy)."""
    t = tokens.astype(np.float32)

    def sigmoid(v):
        return (1.0 / (1.0 + np.exp(-v.astype(np.float32)))).astype(np.float32)

    M_in = (sigmoid(t @ t.T) >= INNER_PRUNE).astype(np.float32)
    deg_tok = 1.0 + M_in.sum(0)
    inv_tok = (1.0 / np.sqrt(deg_tok)).astype(np.float32)
    norm_in = M_in * inv_tok[:, None] * inv_tok[None, :]
    ht1lin = t @ W1
    out_tok1 = norm_in @ ht1lin + ht1lin * (1.0 / deg_tok)[:, None] + b1
    ht1a = np.where(out_tok1 >= 0, out_tok1, NEG_SLOPE * out_tok1).astype(np.float32)
    ht2lin = ht1a @ W2
    out_tok2 = norm_in @ ht2lin + ht2lin * (1.0 / deg_tok)[:, None] + b2
    tok_sum2 = out_tok2.sum(0).astype(np.float32)
    cT1 = inv_tok[:, None] * ht1lin
    cT2p = inv_tok[:, None] * ht1a          # W2 deferred to the head
    cT12 = np.concatenate([cT1, cT2p], axis=1).astype(np.float32)  # [10, 128]
    return cT12, tok_sum2


def _host_graph_prep(src, dst):
    """Ahat = count(src->dst) + I in fp8 DoubleRow pair layout + in-degrees."""
    src = src.astype(np.int64)
    dst = dst.astype(np.int64)
    cnt = np.bincount(src * N + dst, minlength=N * N).reshape(N, N)
    cnt = cnt.astype(np.float32)
    cnt[np.arange(N), np.arange(N)] += 1.0      # fold self-loop term
    # A8[p, u, i, d] = Ahat[(2u+i)*128+p, d]
    A8 = np.ascontiguousarray(
        cnt.reshape(4, 2, 128, N).transpose(2, 0, 1, 3)
    ).astype(FP8)
    indeg = np.bincount(dst, minlength=N).astype(np.float32)
    indeg_w = np.ascontiguousarray(indeg.reshape(8, 128).T)  # [p, t] = indeg[t*128+p]
    return A8, indeg_w


def _build_program(bloc):
    from concourse import bacc, tile, mybir

    fp32 = mybir.dt.float32
    fp32r = mybir.dt.float32r
    bf16 = mybir.dt.bfloat16
    fp8 = mybir.dt.float8e4
    AF = mybir.ActivationFunctionType
    ALU = mybir.AluOpType
    DR = mybir.MatmulPerfMode.DoubleRow

    nc = bacc.Bacc("TRN2", target_bir_lowering=False, debug=True)

    xT_p = nc.declare_dram_parameter("xT", [bloc, F, N], fp32r, isOutput=False)
    A8_p = nc.declare_dram_parameter("A8", [bloc, 128, 4, 2, N], fp8, isOutput=False)
    indeg_p = nc.declare_dram_parameter("indeg", [bloc, 128, 8], fp32, isOutput=False)
    W1_p = nc.declare_dram_parameter("W1", [F, H], fp32r, isOutput=False)
    tokT_p = nc.declare_dram_parameter("tokT", [F, T], fp32r, isOutput=False)
    cT12_p = nc.declare_dram_parameter("cT12", [T, 2 * H], fp32, isOutput=False)
    b1c_p = nc.declare_dram_parameter("b1c", [H, 1], fp32, isOutput=False)
    W2_p = nc.declare_dram_parameter("W2", [H, H], fp32, isOutput=False)
    const64_p = nc.declare_dram_parameter("c64", [H, 1], fp32, isOutput=False)
    Wa_p = nc.declare_dram_parameter("Wa", [H, C], fp32, isOutput=False)
    bat_p = nc.declare_dram_parameter("bat", [bloc, C], fp32, isOutput=False)
    idb_p = nc.declare_dram_parameter("idb", [128, 128], bf16, isOutput=False)
    out_p = nc.declare_dram_parameter("out", [bloc, C], fp32, isOutput=True)
    dinv = [nc.dram_tensor(f"dinv{g}", [N], fp32) for g in range(bloc)]

    with tile.TileContext(nc) as tc:
        with (
            tc.tile_pool(name="const", bufs=1) as cpool,
            tc.tile_pool(name="adj", bufs=2) as apool,
            tc.tile_pool(name="xp", bufs=2) as xpool,
            tc.tile_pool(name="work", bufs=2) as wpool,
            tc.tile_pool(name="ps", bufs=1, space="PSUM") as ps,
        ):
            # ---- constants ----
            W1_t = cpool.tile([F, H], fp32r)
            nc.sync.dma_start(out=W1_t[:], in_=W1_p[:])
            tokT_t = cpool.tile([F, T], fp32r)
            nc.sync.dma_start(out=tokT_t[:], in_=tokT_p[:])
            cT12_t = cpool.tile([T, 2 * H], fp32)
            nc.sync.dma_start(out=cT12_t[:], in_=cT12_p[:])
            cT12_b = cpool.tile([T, 2 * H], bf16)
            nc.vector.tensor_copy(cT12_b[:], cT12_t[:])
            b1c_t = cpool.tile([H, 1], fp32)
            nc.sync.dma_start(out=b1c_t[:], in_=b1c_p[:])
            W2_t = cpool.tile([H, H], fp32)
            nc.sync.dma_start(out=W2_t[:], in_=W2_p[:])
            c64_t = cpool.tile([H, 1], fp32)
            nc.sync.dma_start(out=c64_t[:], in_=const64_p[:])
            Wa_t = cpool.tile([H, C], fp32)
            nc.sync.dma_start(out=Wa_t[:], in_=Wa_p[:])
            bat_t = cpool.tile([bloc, C], fp32)
            nc.sync.dma_start(out=bat_t[:], in_=bat_p[:])
            idb_t = cpool.tile([128, 128], bf16)
            nc.sync.dma_start(out=idb_t[:], in_=idb_p[:])
            ones10 = cpool.tile([T, 1], bf16)
            nc.vector.memset(ones10[:], 1.0)

            SD_T = cpool.tile([H, bloc], fp32)
            sink = cpool.tile([H, N], bf16)

            for g in range(bloc):
                # ---- loads ----
                xT = xpool.tile([F, N], fp32r, tag="xT")
                nc.sync.dma_start(out=xT[:], in_=xT_p[g])
                indeg_t = wpool.tile([128, 8], fp32, tag="indeg")
                nc.sync.dma_start(out=indeg_t[:], in_=indeg_p[g])
                A8_t = apool.tile([128, 4, 2, N], fp8, tag="A")
                nc.sync.dma_start(out=A8_t[:], in_=A8_p[g])

                # ---- M_cr mask [10, 1024] (fp32r matmuls, 512-col halves) ----
                mask_b = wpool.tile([T, N], bf16, tag="mask")
                for hb in range(2):
                    mcr_ps = ps.tile([T, 512], fp32, tag="mcr")
                    nc.tensor.matmul(mcr_ps[:], tokT_t[:],
                                     xT[:, hb * 512:(hb + 1) * 512],
                                     start=True, stop=True)
                    nc.vector.tensor_scalar(mask_b[:, hb * 512:(hb + 1) * 512],
                                            mcr_ps[:], THR_CROSS, None, ALU.is_ge)

                # ---- M_cr column sums -> [128, 8] (per node) ----
                mcrcol_ps = ps.tile([128, 8], fp32, tag="h1")
                for t in range(8):
                    nc.tensor.matmul(mcrcol_ps[:, t:t + 1],
                                     mask_b[:, t * 128:(t + 1) * 128],
                                     ones10[:], start=(t == 0), stop=(t == 7))

                # ---- deg / inv (node-major), invrep via DRAM + broadcast ----
                dn = wpool.tile([128, 8], fp32, tag="dn")
                nc.vector.tensor_tensor(dn[:], indeg_t[:], mcrcol_ps[:], ALU.add)
                nc.vector.tensor_scalar_add(dn[:], dn[:], 1.0)
                inv2c = wpool.tile([128, 8], fp32, tag="inv2c")
                nc.vector.reciprocal(inv2c[:], dn[:])
                invc = wpool.tile([128, 8], fp32, tag="invc")
                nc.scalar.sqrt(invc[:], inv2c[:])
                nc.sync.dma_start(out=dinv[g].rearrange("(t p) -> p t", p=128),
                                  in_=invc[:])
                invrow = wpool.tile([1, N], fp32, tag="invrow")
                nc.sync.dma_start(out=invrow[:],
                                  in_=dinv[g].rearrange("(o n) -> o n", o=1))
                invrep = xpool.tile([H, N], fp32, tag="invrep")
                nc.gpsimd.partition_broadcast(invrep[:], invrow[:])

                # ---- h1 node-major fp8 = fp8(inv * (x @ W1)) ----
                h1b = xpool.tile([128, 8, H], fp8, tag="h1b")
                for t in range(8):
                    hps = ps.tile([128, H], fp32, tag="h1")
                    nc.tensor.matmul(hps[:], xT[:, t * 128:(t + 1) * 128], W1_t[:],
                                     start=True, stop=True)
                    nc.scalar.activation(h1b[:, t, :], hps[:], AF.Copy,
                                         scale=invc[:, t:t + 1])

                # ---- cross terms crsT [128=2H, 1024] (col halves in PSUM) ----
                crsT_a = ps.tile([2 * H, 512], fp32, tag="crsT")
                nc.tensor.matmul(crsT_a[:], cT12_b[:], mask_b[:, 0:512],
                                 start=True, stop=True)
                crsT_b = ps.tile([2 * H, 512], fp32, tag="crsT")
                nc.tensor.matmul(crsT_b[:], cT12_b[:], mask_b[:, 512:1024],
                                 start=True, stop=True)
                crsT_sb = xpool.tile([2 * H, N], fp32, tag="crsT_sb")
                nc.vector.tensor_copy(crsT_sb[:, 0:512], crsT_a[:])
                nc.vector.tensor_copy(crsT_sb[:, 512:1024], crsT_b[:])

                # ---- layer 1: yT = h1^T @ Ahat (fp8 DoubleRow) ----
                yT_ps = ps.tile([H, N], fp32, tag="agg")
                for hf in range(2):
                    for u in range(4):
                        nc.tensor.matmul(
                            yT_ps[:, hf * 512:(hf + 1) * 512],
                            h1b[:, 2 * u:2 * u + 2, :],
                            A8_t[:, u, :, hf * 512:(hf + 1) * 512],
                            start=(u == 0), stop=(u == 3), perf_mode=DR)

                # ---- hnT = lrelu((yT + crs1T) * invrep + b1) ----
                tmp = xpool.tile([H, N], fp32, tag="tmp")
                nc.vector.tensor_tensor(tmp[:], yT_ps[:], crsT_sb[0:H, :], ALU.add)
                nc.vector.tensor_tensor(tmp[:], tmp[:], invrep[:], ALU.mult)
                hnT = xpool.tile([H, N], fp32, tag="hnT")
                nc.scalar.activation(hnT[:], tmp[:], AF.Lrelu,
                                     bias=b1c_t[:], alpha=NEG_SLOPE)

                # ---- g2 = fp8(inv * hn), node-major via PE transposes ----
                g2Tb = xpool.tile([H, N], bf16, tag="g2Tb")
                nc.vector.tensor_tensor(g2Tb[:], hnT[:], invrep[:], ALU.mult)
                g2b = xpool.tile([128, 8, H], fp8, tag="g2b")
                for t in range(8):
                    tps = ps.tile([128, H], bf16, tag="trans")
                    nc.tensor.transpose(tps[:], g2Tb[:, t * 128:(t + 1) * 128],
                                        idb_t[0:H, 0:H])
                    nc.scalar.copy(g2b[:, t, :], tps[:])

                # ---- layer 2: agg2T = g2^T @ Ahat (fp8 DoubleRow) ----
                a2_ps = ps.tile([H, N], fp32, tag="agg")
                for hf in range(2):
                    for u in range(4):
                        nc.tensor.matmul(
                            a2_ps[:, hf * 512:(hf + 1) * 512],
                            g2b[:, 2 * u:2 * u + 2, :],
                            A8_t[:, u, :, hf * 512:(hf + 1) * 512],
                            start=(u == 0), stop=(u == 3), perf_mode=DR)

                # ---- sd = sum_n ((agg2T + crs2T) * invrep)[:, n] ----
                m1 = xpool.tile([H, N], fp32, tag="m1")
                nc.vector.tensor_tensor(m1[:], a2_ps[:], crsT_sb[H:2 * H, :], ALU.add)
                nc.vector.tensor_tensor(m1[:], m1[:], invrep[:], ALU.mult)
                nc.vector.tensor_reduce(SD_T[:, g:g + 1], m1[:],
                                        mybir.AxisListType.X, ALU.add)

            # ---- batched head ----
            emb_ps = ps.tile([H, bloc], fp32, tag="trans")
            nc.tensor.matmul(emb_ps[:], W2_t[:], SD_T[:], start=True, stop=True)
            embT = cpool.tile([H, bloc], fp32)
            nc.vector.tensor_scalar(embT[:], emb_ps[:], c64_t[:], None, ALU.add)
            lg_ps = ps.tile([bloc, C], fp32, tag="trans")
            nc.tensor.matmul(lg_ps[:], embT[:], Wa_t[:], start=True, stop=True)
            lg = cpool.tile([bloc, C], fp32)
            nc.vector.tensor_tensor(lg[:], lg_ps[:], bat_t[:], ALU.add)
            mx = cpool.tile([bloc, 1], fp32)
            nc.vector.tensor_reduce(mx[:], lg[:], mybir.AxisListType.X, ALU.max)
            nmx = cpool.tile([bloc, 1], fp32)
            nc.vector.tensor_scalar_mul(nmx[:], mx[:], -1.0)
            ex = cpool.tile([bloc, C], fp32)
            nc.scalar.activation(ex[:], lg[:], AF.Exp, bias=nmx[:])
            sm = cpool.tile([bloc, 1], fp32)
            nc.vector.tensor_reduce(sm[:], ex[:], mybir.AxisListType.X, ALU.add)
            rs = cpool.tile([bloc, 1], fp32)
            nc.vector.reciprocal(rs[:], sm[:])
            outt = cpool.tile([bloc, C], fp32)
            nc.vector.tensor_scalar(outt[:], ex[:], rs[:], None, ALU.mult)
            nc.sync.dma_start(out=out_p[:], in_=outt[:])

    nc.compile()
    return nc


def _get_program(bloc=BLOC):
    if bloc not in _CACHE:
        _CACHE[bloc] = _build_program(bloc)
    return _CACHE[bloc]


def build_in_maps(x, tokens, W1, b1, W2, b2, Wa, ba, edge_src, edge_dst,
                  ncores=NCORES, bloc=BLOC):
    x = np.asarray(x, np.float32)
    cT12, tok_sum2 = _token_constants(
        np.asarray(tokens, np.float32), np.asarray(W1, np.float32),
        np.asarray(b1, np.float32), np.asarray(W2, np.float32),
        np.asarray(b2, np.float32), np.asarray(Wa, np.float32),
        np.asarray(ba, np.float32))
    const64 = (N * np.asarray(b2, np.float32) + tok_sum2).reshape(H, 1)
    shared = {
        "W1": np.asarray(W1, np.float32),
        "tokT": np.ascontiguousarray(np.asarray(tokens, np.float32).T),
        "cT12": cT12,
        "b1c": np.asarray(b1, np.float32).reshape(H, 1),
        "W2": np.asarray(W2, np.float32),
        "c64": const64,
        "Wa": (np.asarray(Wa, np.float32) / float(T + N)),
        "bat": np.tile(np.asarray(ba, np.float32)[None, :], (bloc, 1)),
        "idb": np.eye(128, dtype=np.float32).astype(ml_dtypes.bfloat16),
    }
    in_maps = []
    for c in range(ncores):
        A8 = np.zeros((bloc, 128, 4, 2, N), FP8)
        indeg_w = np.zeros((bloc, 128, 8), np.float32)
        xTl = np.zeros((bloc, F, N), np.float32)
        for g in range(bloc):
            gi = c * bloc + g
            A8[g], indeg_w[g] = _host_graph_prep(
                np.asarray(edge_src[gi]), np.asarray(edge_dst[gi]))
            xTl[g] = x[gi].T
        m = dict(shared)
        m["xT"] = xTl
        m["A8"] = A8
        m["indeg"] = indeg_w
        in_maps.append(m)
    return in_maps


def kernel(x, tokens, W1, b1, W2, b2, Wa, ba, edge_src, edge_dst):
    from concourse.bass_utils import run_bass_kernel_spmd

    nc = _get_program()
    in_maps = build_in_maps(x, tokens, W1, b1, W2, b2, Wa, ba, edge_src, edge_dst)
    res = run_bass_kernel_spmd(nc, in_maps, list(range(NCORES)))
    out = np.concatenate([res.results[c]["out"] for c in range(NCORES)], axis=0)
    return out.astype(np.float32)
